# revision 13
# baseline (speedup 1.0000x reference)
"""im2col (3x3, SAME zero padding) kernel for Trainium2.

Full op: x (16, 64, 128, 128) f32 -> out (16, 128, 128, 64, 3, 3) f32 with
    out[b, h, w, c, i, j] = pad(x)[b, c, h + i, w + j]   (pad = 1 px zeros)

Sharding: data-parallel over batch. 8 cores x 2 batches each, no
cross-device communication.

The op is pure data movement: per core 8.4 MB HBM read + 75.5 MB HBM
write, against a measured ~355 GB/s per-core HBM DMA ceiling -> ~242 us
practical floor including the read/write mix. The kernel structure exists
only to keep the store DMA queue saturated:

  1. Loads ride the ACT (scalar) HWDGE ring, stores the SP (sync) ring.
     On one ring they serialize (FIFO per ring) and cost ~30 us.
  2. Stream x[b] in 64-row chunks into SBUF (64c, 66 rows x 130) with
     zero pad columns/halo rows materialized, so all nine shifted reads
     are plain AP offsets.
  3. Per padded row, 3 TensorE transposes (lhsT = (64, 128) row window at
     w-offset j, rhs = 64x64 identity) -> PSUM (128w, [j][c]).
  4. Each PSUM tile is scattered straight into the interleaved output
     staging buffer: for i in 0..2, one strided (p, c, j) tensor_copy
     into osb[h = row - i][:, c, i, j]. No intermediate staging stage;
     copies rotate over DVE/DVE/DVE/ACT (GPSIMD has no PSUM port and
     contends with DVE on a shared SBUF port pair).
  5. One ~1.2 MB DMA stores G=4 output rows (contiguous 2304 B per
     (h, w)) as soon as their last contributing input row is copied;
     8 osb buffers keep the store queue deep.
"""

import sys

for _p in ("/opt/trn_rl_repo", "/root/.axon_site/_ro/trn_rl_repo"):
    if _p not in sys.path:
        sys.path.append(_p)

import numpy as np

import concourse.bacc as bacc
import concourse.mybir as mybir
from concourse import bass_utils, masks
from concourse.tile import TileContext

F32 = mybir.dt.float32

# Problem shape (hardcoded; the grading harness provides exactly this).
B, C, H, W = 16, 64, 128, 128
KS = 3  # kernel size
N_CORES = 8
B_LOC = B // N_CORES  # batches per core

WP = W + 2  # padded row length
CH = 32  # h-chunk size
CHP = CH + 2  # padded rows per chunk
G = 4  # output rows per store DMA


def _build_kernel(n_b: int = B_LOC, repeat: int = 1, g: int = G, ch: int = CH,
                  xin_bufs: int = 2, xt_bufs: int = 2, ps_bufs: int = 4,
                  osb_bufs: int = 0, dma_split: bool = False,
                  load_act: bool = False, asm: str = "vga", ps_copy: str = "va"):
    nc = bacc.Bacc("TRN2", target_bir_lowering=False, debug=False)

    x = nc.dram_tensor("x", (n_b, C, H, W), F32, kind="ExternalInput")
    out = nc.dram_tensor("out", (n_b, H, W, C, KS, KS), F32, kind="ExternalOutput")
    x_ap = x.ap()
    out_ap = out.ap()

    with TileContext(nc) as tc:
        with (
            tc.tile_pool(name="const", bufs=1) as const_pool,
            tc.tile_pool(name="xin", bufs=xin_bufs) as xin_pool,
            tc.tile_pool(name="xt", bufs=xt_bufs) as xt_pool,
            tc.tile_pool(name="ps", bufs=ps_bufs, space="PSUM") as psum_pool,
            tc.tile_pool(
                name="osb", bufs=(osb_bufs or (4 if g <= 4 else 3))
            ) as out_pool,
        ):
            ident = const_pool.tile([C, C], F32)
            masks.make_identity(nc, ident)

            eng_map = {
                "v": nc.vector.tensor_copy,
                "g": nc.gpsimd.tensor_copy,
                "a": nc.scalar.copy,
            }
            copy_engines = [eng_map[c] for c in ps_copy]
            asm_engines = [eng_map[c] for c in asm]

            chp = ch + 2
            for _rep in range(repeat):
              for b in range(n_b):
                for h0 in range(0, H, ch):
                    # ---- load chunk: padded rows h0 .. h0+CHP-1 (global
                    # unpadded rows h0-1 .. h0+CH) ----
                    xin = xin_pool.tile([C, chp * WP], F32)
                    xin_r = xin.rearrange("p (r q) -> p r q", q=WP)
                    # zero pad columns (w = -1 and w = W)
                    nc.vector.memset(xin_r[:, :, 0:1], 0.0)
                    nc.vector.memset(xin_r[:, :, WP - 1 : WP], 0.0)
                    g_lo = h0 - 1
                    lo = 0
                    n_rows = chp
                    if g_lo < 0:  # top halo row is out of image -> zeros
                        nc.vector.memset(xin_r[:, 0:1, :], 0.0)
                        g_lo, lo, n_rows = 0, 1, n_rows - 1
                    if h0 + ch + 1 > H:  # bottom halo row -> zeros
                        nc.vector.memset(xin_r[:, chp - 1 : chp, :], 0.0)
                        n_rows -= 1
                    ld_eng = nc.scalar if load_act else nc.sync
                    ld_eng.dma_start(
                        out=xin_r[:, lo : lo + n_rows, 1 : W + 1],
                        in_=x_ap[b, :, g_lo : g_lo + n_rows, :],
                    )

                    # ---- transpose every padded row, 3 w-shifts each ----
                    xt = xt_pool.tile([W, chp * KS * C], F32)
                    for li in range(chp):
                        ps = psum_pool.tile([W, KS * C], F32)
                        for j in range(KS):
                            nc.tensor.transpose(
                                ps[:, j * C : (j + 1) * C],
                                xin_r[:, li, j : j + W],
                                ident,
                            )
                        # stage PSUM -> SBUF (rotate engines)
                        copy_engines[li % len(copy_engines)](
                            xt[:, li * KS * C : (li + 1) * KS * C], ps
                        )

                    # ---- assemble + store, G output rows per DMA ----
                    xt_r = xt.rearrange("p (r j c) -> p r j c", j=KS, c=C)
                    for hg in range(0, ch, g):
                        osb = out_pool.tile([W, g * C * KS * KS], F32)
                        # dims: (p, g, i, c, j) so copies see (p, i, c)
                        osb_v = osb.rearrange(
                            "p (g c i j) -> p g i c j", g=g, c=C, i=KS, j=KS
                        )
                        for hs in range(g):
                            hl = hg + hs  # chunk-local output row
                            for j in range(KS):
                                src = xt_r[:, hl : hl + KS, j, :]  # (p, i, c)
                                dst = osb_v[:, hs, :, :, j]  # (p, i, c)
                                asm_engines[(hs * KS + j) % len(asm_engines)](
                                    dst, src
                                )
                        st_eng = (
                            nc.scalar if dma_split and (hg // g) % 2 else nc.sync
                        )
                        st_eng.dma_start(
                            out=out_ap[b].rearrange("h w c i j -> w h (c i j)")[
                                :, h0 + hg : h0 + hg + g, :
                            ],
                            in_=osb.rearrange("p (g f) -> p g f", f=C * KS * KS),
                        )

    nc.compile()
    return nc


def _build_kernel_v3(n_b: int = B_LOC, repeat: int = 1, g: int = G, ch: int = CH,
                     xin_bufs: int = 2, ps_bufs: int = 8, osb_bufs: int = 8,
                     load_act: bool = True, asm: str = "vva"):
    """v3: transpose PSUM tiles are copied straight into the interleaved
    output staging buffer (osb), one strided copy per (padded row P ->
    output row h) pair; the xt staging stage is gone.

    Padded row P feeds output rows h in {P-2, P-1, P} (i = h's kernel row
    index = P - h). The copy for (P, h) moves ps (p, j, c) -> osb[h][:,
    c, i, j] as one 3D-AP tensor_copy. A store group of g rows is DMA'd
    once its last contributing row (P = hg_end + 1) has been copied.
    """
    nc = bacc.Bacc("TRN2", target_bir_lowering=False, debug=False)

    x = nc.dram_tensor("x", (n_b, C, H, W), F32, kind="ExternalInput")
    out = nc.dram_tensor("out", (n_b, H, W, C, KS, KS), F32, kind="ExternalOutput")
    x_ap = x.ap()
    out_ap = out.ap()

    with TileContext(nc) as tc:
        with (
            tc.tile_pool(name="const", bufs=1) as const_pool,
            tc.tile_pool(name="xin", bufs=xin_bufs) as xin_pool,
            tc.tile_pool(name="ps", bufs=ps_bufs, space="PSUM") as psum_pool,
            tc.tile_pool(name="osb", bufs=osb_bufs) as out_pool,
        ):
            ident = const_pool.tile([C, C], F32)
            masks.make_identity(nc, ident)

            eng_map = {"v": nc.vector.tensor_copy, "a": nc.scalar.copy}
            asm_engines = [eng_map[c] for c in asm]
            n_asm = 0

            chp = ch + 2
            for _rep in range(repeat):
              for b in range(n_b):
                for h0 in range(0, H, ch):
                    # ---- load chunk: padded rows h0 .. h0+chp-1 ----
                    xin = xin_pool.tile([C, chp * WP], F32)
                    xin_r = xin.rearrange("p (r q) -> p r q", q=WP)
                    nc.vector.memset(xin_r[:, :, 0:1], 0.0)
                    nc.vector.memset(xin_r[:, :, WP - 1 : WP], 0.0)
                    g_lo = h0 - 1
                    lo = 0
                    n_rows = chp
                    if g_lo < 0:
                        nc.vector.memset(xin_r[:, 0:1, :], 0.0)
                        g_lo, lo, n_rows = 0, 1, n_rows - 1
                    if h0 + ch + 1 > H:
                        nc.vector.memset(xin_r[:, chp - 1 : chp, :], 0.0)
                        n_rows -= 1
                    ld_eng = nc.scalar if load_act else nc.sync
                    ld_eng.dma_start(
                        out=xin_r[:, lo : lo + n_rows, 1 : W + 1],
                        in_=x_ap[b, :, g_lo : g_lo + n_rows, :],
                    )

                    # ---- per padded row: transpose, then scatter into osb ----
                    osb_tiles = {}  # chunk-local group idx -> (tile, view)

                    def get_group(gi):
                        if gi not in osb_tiles:
                            t = out_pool.tile([W, g * C * KS * KS], F32)
                            # dims (p, row-in-group, c, i, j)
                            v = t.rearrange(
                                "p (s c i j) -> p s c i j", s=g, c=C, i=KS, j=KS
                            )
                            osb_tiles[gi] = (t, v)
                        return osb_tiles[gi]

                    def store_group(gi):
                        t, _ = osb_tiles.pop(gi)
                        nc.sync.dma_start(
                            out=out_ap[b].rearrange("h w c i j -> w h (c i j)")[
                                :, h0 + gi * g : h0 + (gi + 1) * g, :
                            ],
                            in_=t.rearrange("p (s f) -> p s f", f=C * KS * KS),
                        )

                    for li in range(chp):
                        ps = psum_pool.tile([W, KS * C], F32)
                        for j in range(KS):
                            nc.tensor.transpose(
                                ps[:, j * C : (j + 1) * C],
                                xin_r[:, li, j : j + W],
                                ident,
                            )
                        ps_v = ps.rearrange("p (j c) -> p j c", j=KS)
                        src = ps_v.rearrange("p j c -> p c j")
                        for i in range(KS):
                            hl = li - i  # chunk-local output row
                            if not (0 <= hl < ch):
                                continue
                            gi, s = divmod(hl, g)
                            _, v = get_group(gi)
                            # dst (p, c, j) over osb[h] at fixed i
                            dst = v[:, s, :, i, :]
                            asm_engines[n_asm % len(asm_engines)](dst, src)
                            n_asm += 1
                        # group gi's last feeder row is li = (gi+1)*g + 1
                        if li >= g + 1 and (li - 1) % g == 0:
                            store_group((li - 1) // g - 1)
                    assert not osb_tiles, osb_tiles

    nc.compile()
    return nc


def _build_kernel_v5(n_b: int = B_LOC, repeat: int = 1, g: int = G, ch: int = CH,
                     xin_bufs: int = 2, ps_bufs: int = 8, osb_bufs: int = 8,
                     load_act: bool = True, asm: str = "vva"):
    """v5: v3's 64-wide transpose/copy pipeline, but the chunk load is a
    single 128-partition (b c) DMA (full 16-engine rate); transposes for
    batch b read lhsT at partition offset b*64.
    """
    assert n_b == 2, n_b
    nc = bacc.Bacc("TRN2", target_bir_lowering=False, debug=False)

    x = nc.dram_tensor("x", (n_b, C, H, W), F32, kind="ExternalInput")
    out = nc.dram_tensor("out", (n_b, H, W, C, KS, KS), F32, kind="ExternalOutput")
    x_bc = x.ap().rearrange("b c h w -> (b c) h w")
    out_ap = out.ap()
    BC = n_b * C

    with TileContext(nc) as tc:
        with (
            tc.tile_pool(name="const", bufs=1) as const_pool,
            tc.tile_pool(name="xin", bufs=xin_bufs) as xin_pool,
            tc.tile_pool(name="ps", bufs=ps_bufs, space="PSUM") as psum_pool,
            tc.tile_pool(name="osb", bufs=osb_bufs) as out_pool,
        ):
            # identity block replicated on both partition halves so the
            # rhs base_partition can match lhsT's (b*C offset)
            ident2 = const_pool.tile([BC, C], F32)
            masks.make_identity(nc, ident2[0:C, :])
            masks.make_identity(nc, ident2[C:BC, :])

            eng_map = {"v": nc.vector.tensor_copy, "a": nc.scalar.copy}
            asm_engines = [eng_map[c] for c in asm]
            n_asm = 0

            chp = ch + 2
            for _rep in range(repeat):
                for h0 in range(0, H, ch):
                    xin = xin_pool.tile([BC, chp * WP], F32)
                    xin_r = xin.rearrange("p (r q) -> p r q", q=WP)
                    nc.vector.memset(xin_r[:, :, 0:1], 0.0)
                    nc.vector.memset(xin_r[:, :, WP - 1 : WP], 0.0)
                    g_lo = h0 - 1
                    lo = 0
                    n_rows = chp
                    if g_lo < 0:
                        nc.vector.memset(xin_r[:, 0:1, :], 0.0)
                        g_lo, lo, n_rows = 0, 1, n_rows - 1
                    if h0 + ch + 1 > H:
                        nc.vector.memset(xin_r[:, chp - 1 : chp, :], 0.0)
                        n_rows -= 1
                    ld_eng = nc.scalar if load_act else nc.sync
                    ld_eng.dma_start(
                        out=xin_r[:, lo : lo + n_rows, 1 : W + 1],
                        in_=x_bc[:, g_lo : g_lo + n_rows, :],
                    )

                    osb_tiles = {}

                    def get_group(b, gi):
                        key = (b, gi)
                        if key not in osb_tiles:
                            t = out_pool.tile([W, g * C * KS * KS], F32)
                            v = t.rearrange(
                                "p (s c i j) -> p s c i j", s=g, c=C, i=KS, j=KS
                            )
                            osb_tiles[key] = (t, v)
                        return osb_tiles[key]

                    def store_group(b, gi):
                        t, _ = osb_tiles.pop((b, gi))
                        nc.sync.dma_start(
                            out=out_ap[b].rearrange("h w c i j -> w h (c i j)")[
                                :, h0 + gi * g : h0 + (gi + 1) * g, :
                            ],
                            in_=t.rearrange("p (s f) -> p s f", f=C * KS * KS),
                        )

                    for li in range(chp):
                        for b in range(n_b):
                            ps = psum_pool.tile([W, KS * C], F32)
                            for j in range(KS):
                                nc.tensor.transpose(
                                    ps[:, j * C : (j + 1) * C],
                                    xin_r[b * C : (b + 1) * C, li, j : j + W],
                                    ident2[b * C : (b + 1) * C, :],
                                )
                            src = ps.rearrange("p (j c) -> p j c", j=KS).rearrange(
                                "p j c -> p c j"
                            )
                            for i in range(KS):
                                hl = li - i
                                if not (0 <= hl < ch):
                                    continue
                                gi, s = divmod(hl, g)
                                _, v = get_group(b, gi)
                                dst = v[:, s, :, i, :]
                                asm_engines[n_asm % len(asm_engines)](dst, src)
                                n_asm += 1
                        if li >= g + 1 and (li - 1) % g == 0:
                            gi = (li - 1) // g - 1
                            for b in range(n_b):
                                store_group(b, gi)
                    assert not osb_tiles, osb_tiles

    nc.compile()
    return nc


def _build_kernel_v4(n_b: int = B_LOC, repeat: int = 1, g: int = G, ch: int = CH,
                     xin_bufs: int = 2, ps_bufs: int = 8, osb_bufs: int = 8,
                     load_act: bool = True, asm: str = "vva"):
    """v4: like v3 (PSUM -> osb direct), but both batches ride one
    128-partition pipeline: the chunk load is a single full-rate DMA with
    partition dim (b c), and each transpose is 128-wide (contraction over
    (b c)), halving TensorE work. Copies then slice the batch halves out
    of PSUM into per-batch osb groups.
    """
    assert n_b == 2, n_b
    nc = bacc.Bacc("TRN2", target_bir_lowering=False, debug=False)

    x = nc.dram_tensor("x", (n_b, C, H, W), F32, kind="ExternalInput")
    out = nc.dram_tensor("out", (n_b, H, W, C, KS, KS), F32, kind="ExternalOutput")
    x_bc = x.ap().rearrange("b c h w -> (b c) h w")
    out_ap = out.ap()
    BC = n_b * C  # 128

    with TileContext(nc) as tc:
        with (
            tc.tile_pool(name="const", bufs=1) as const_pool,
            tc.tile_pool(name="xin", bufs=xin_bufs) as xin_pool,
            tc.tile_pool(name="ps", bufs=ps_bufs, space="PSUM") as psum_pool,
            tc.tile_pool(name="osb", bufs=osb_bufs) as out_pool,
        ):
            ident = const_pool.tile([BC, BC], F32)
            masks.make_identity(nc, ident)

            eng_map = {"v": nc.vector.tensor_copy, "a": nc.scalar.copy}
            asm_engines = [eng_map[c] for c in asm]
            n_asm = 0

            chp = ch + 2
            for _rep in range(repeat):
                for h0 in range(0, H, ch):
                    # ---- one 128-partition load for both batches ----
                    xin = xin_pool.tile([BC, chp * WP], F32)
                    xin_r = xin.rearrange("p (r q) -> p r q", q=WP)
                    nc.vector.memset(xin_r[:, :, 0:1], 0.0)
                    nc.vector.memset(xin_r[:, :, WP - 1 : WP], 0.0)
                    g_lo = h0 - 1
                    lo = 0
                    n_rows = chp
                    if g_lo < 0:
                        nc.vector.memset(xin_r[:, 0:1, :], 0.0)
                        g_lo, lo, n_rows = 0, 1, n_rows - 1
                    if h0 + ch + 1 > H:
                        nc.vector.memset(xin_r[:, chp - 1 : chp, :], 0.0)
                        n_rows -= 1
                    ld_eng = nc.scalar if load_act else nc.sync
                    ld_eng.dma_start(
                        out=xin_r[:, lo : lo + n_rows, 1 : W + 1],
                        in_=x_bc[:, g_lo : g_lo + n_rows, :],
                    )

                    osb_tiles = {}  # (b, chunk-local group idx) -> (tile, view)

                    def get_group(b, gi):
                        key = (b, gi)
                        if key not in osb_tiles:
                            t = out_pool.tile([W, g * C * KS * KS], F32)
                            v = t.rearrange(
                                "p (s c i j) -> p s c i j", s=g, c=C, i=KS, j=KS
                            )
                            osb_tiles[key] = (t, v)
                        return osb_tiles[key]

                    def store_group(b, gi):
                        t, _ = osb_tiles.pop((b, gi))
                        nc.sync.dma_start(
                            out=out_ap[b].rearrange("h w c i j -> w h (c i j)")[
                                :, h0 + gi * g : h0 + (gi + 1) * g, :
                            ],
                            in_=t.rearrange("p (s f) -> p s f", f=C * KS * KS),
                        )

                    for li in range(chp):
                        ps = psum_pool.tile([W, KS * BC], F32)
                        for j in range(KS):
                            nc.tensor.transpose(
                                ps[:, j * BC : (j + 1) * BC],
                                xin_r[:, li, j : j + W],
                                ident,
                            )
                        ps_v = ps.rearrange("p (j bc) -> p j bc", j=KS)
                        for i in range(KS):
                            hl = li - i  # chunk-local output row
                            if not (0 <= hl < ch):
                                continue
                            gi, s = divmod(hl, g)
                            for b in range(n_b):
                                _, v = get_group(b, gi)
                                dst = v[:, s, :, i, :]
                                src = ps_v[:, :, b * C : (b + 1) * C].rearrange(
                                    "p j c -> p c j"
                                )
                                asm_engines[n_asm % len(asm_engines)](dst, src)
                                n_asm += 1
                        if li >= g + 1 and (li - 1) % g == 0:
                            gi = (li - 1) // g - 1
                            for b in range(n_b):
                                store_group(b, gi)
                    assert not osb_tiles, osb_tiles

    nc.compile()
    return nc


FINAL_KW = dict(
    g=4, ch=64, xin_bufs=2, ps_bufs=8, osb_bufs=8, load_act=True, asm="vvva"
)

_NC_CACHE = {}


def _get_nc(n_b: int):
    if n_b not in _NC_CACHE:
        _NC_CACHE[n_b] = _build_kernel_v3(n_b, **FINAL_KW)
    return _NC_CACHE[n_b]


def run_spmd(x: np.ndarray, **kwargs) -> bass_utils.BassKernelResults:
    """Run the SPMD kernel on 8 cores; returns raw BassKernelResults."""
    x = np.ascontiguousarray(np.asarray(x, dtype=np.float32))
    assert x.shape == (B, C, H, W), x.shape
    nc = _get_nc(B_LOC)
    in_maps = [
        {"x": x[i * B_LOC : (i + 1) * B_LOC]} for i in range(N_CORES)
    ]
    return bass_utils.run_bass_kernel_spmd(
        nc, in_maps, core_ids=list(range(N_CORES)), **kwargs
    )


def kernel(x: np.ndarray) -> np.ndarray:
    res = run_spmd(x)
    return np.concatenate([r["out"] for r in res.results], axis=0)



# revision 20
# speedup vs baseline: 1.0527x; 1.0527x over previous
"""im2col (3x3, SAME zero padding) kernel for Trainium2.

Full op: x (16, 64, 128, 128) f32 -> out (16, 128, 128, 64, 3, 3) f32 with
    out[b, h, w, c, i, j] = pad(x)[b, c, h + i, w + j]   (pad = 1 px zeros)

Sharding: data-parallel over batch. 8 cores x 2 batches each, no
cross-device communication.

The op is pure data movement: per core 8.4 MB HBM read + 75.5 MB HBM
write, against a measured ~355 GB/s per-core HBM DMA ceiling -> ~242 us
practical floor including the read/write mix. The kernel structure exists
only to keep the store DMA queue saturated:

  1. Loads ride the ACT (scalar) HWDGE ring, stores the SP (sync) ring.
     On one ring they serialize (FIFO per ring) and cost ~30 us.
  2. Stream x[b] in 64-row chunks into SBUF (64c, 66 rows x 130) with
     zero pad columns/halo rows materialized, so all nine shifted reads
     are plain AP offsets.
  3. Per padded row, 3 TensorE transposes (lhsT = (64, 128) row window at
     w-offset j, rhs = 64x64 identity) -> PSUM (128w, [j][c]).
  4. Each PSUM tile is scattered straight into the interleaved output
     staging buffer: for i in 0..2, one strided (p, c, j) tensor_copy
     into osb[h = row - i][:, c, i, j]. No intermediate staging stage.
     ALL copies run on DVE: SP and ACT are the two HWDGE sequencers
     (stores ride SP's ring, loads ACT's), and any copy work on ACT
     stalls its descriptor generation and throttles DMA (~12%, measured)
     — so both are kept free for DMA issue. GPSIMD has no PSUM port and
     contends with DVE on a shared SBUF port pair, so it idles too.
  5. One ~1.2 MB DMA stores G=4 output rows (contiguous 2304 B per
     (h, w)) as soon as their last contributing input row is copied;
     8 osb buffers keep the store queue deep.
"""

import sys

for _p in ("/opt/trn_rl_repo", "/root/.axon_site/_ro/trn_rl_repo"):
    if _p not in sys.path:
        sys.path.append(_p)

import numpy as np

import concourse.bacc as bacc
import concourse.mybir as mybir
from concourse import bass_utils, masks
from concourse.tile import TileContext

F32 = mybir.dt.float32

# Problem shape (hardcoded; the grading harness provides exactly this).
B, C, H, W = 16, 64, 128, 128
KS = 3  # kernel size
N_CORES = 8
B_LOC = B // N_CORES  # batches per core

WP = W + 2  # padded row length
CH = 32  # h-chunk size
CHP = CH + 2  # padded rows per chunk
G = 4  # output rows per store DMA


def _build_kernel(n_b: int = B_LOC, repeat: int = 1, g: int = G, ch: int = CH,
                  xin_bufs: int = 2, xt_bufs: int = 2, ps_bufs: int = 4,
                  osb_bufs: int = 0, dma_split: bool = False,
                  load_act: bool = False, asm: str = "vga", ps_copy: str = "va"):
    nc = bacc.Bacc("TRN2", target_bir_lowering=False, debug=False)

    x = nc.dram_tensor("x", (n_b, C, H, W), F32, kind="ExternalInput")
    out = nc.dram_tensor("out", (n_b, H, W, C, KS, KS), F32, kind="ExternalOutput")
    x_ap = x.ap()
    out_ap = out.ap()

    with TileContext(nc) as tc:
        with (
            tc.tile_pool(name="const", bufs=1) as const_pool,
            tc.tile_pool(name="xin", bufs=xin_bufs) as xin_pool,
            tc.tile_pool(name="xt", bufs=xt_bufs) as xt_pool,
            tc.tile_pool(name="ps", bufs=ps_bufs, space="PSUM") as psum_pool,
            tc.tile_pool(
                name="osb", bufs=(osb_bufs or (4 if g <= 4 else 3))
            ) as out_pool,
        ):
            ident = const_pool.tile([C, C], F32)
            masks.make_identity(nc, ident)

            eng_map = {
                "v": nc.vector.tensor_copy,
                "g": nc.gpsimd.tensor_copy,
                "a": nc.scalar.copy,
            }
            copy_engines = [eng_map[c] for c in ps_copy]
            asm_engines = [eng_map[c] for c in asm]

            chp = ch + 2
            for _rep in range(repeat):
              for b in range(n_b):
                for h0 in range(0, H, ch):
                    # ---- load chunk: padded rows h0 .. h0+CHP-1 (global
                    # unpadded rows h0-1 .. h0+CH) ----
                    xin = xin_pool.tile([C, chp * WP], F32)
                    xin_r = xin.rearrange("p (r q) -> p r q", q=WP)
                    # zero pad columns (w = -1 and w = W)
                    nc.vector.memset(xin_r[:, :, 0:1], 0.0)
                    nc.vector.memset(xin_r[:, :, WP - 1 : WP], 0.0)
                    g_lo = h0 - 1
                    lo = 0
                    n_rows = chp
                    if g_lo < 0:  # top halo row is out of image -> zeros
                        nc.vector.memset(xin_r[:, 0:1, :], 0.0)
                        g_lo, lo, n_rows = 0, 1, n_rows - 1
                    if h0 + ch + 1 > H:  # bottom halo row -> zeros
                        nc.vector.memset(xin_r[:, chp - 1 : chp, :], 0.0)
                        n_rows -= 1
                    ld_eng = nc.scalar if load_act else nc.sync
                    ld_eng.dma_start(
                        out=xin_r[:, lo : lo + n_rows, 1 : W + 1],
                        in_=x_ap[b, :, g_lo : g_lo + n_rows, :],
                    )

                    # ---- transpose every padded row, 3 w-shifts each ----
                    xt = xt_pool.tile([W, chp * KS * C], F32)
                    for li in range(chp):
                        ps = psum_pool.tile([W, KS * C], F32)
                        for j in range(KS):
                            nc.tensor.transpose(
                                ps[:, j * C : (j + 1) * C],
                                xin_r[:, li, j : j + W],
                                ident,
                            )
                        # stage PSUM -> SBUF (rotate engines)
                        copy_engines[li % len(copy_engines)](
                            xt[:, li * KS * C : (li + 1) * KS * C], ps
                        )

                    # ---- assemble + store, G output rows per DMA ----
                    xt_r = xt.rearrange("p (r j c) -> p r j c", j=KS, c=C)
                    for hg in range(0, ch, g):
                        osb = out_pool.tile([W, g * C * KS * KS], F32)
                        # dims: (p, g, i, c, j) so copies see (p, i, c)
                        osb_v = osb.rearrange(
                            "p (g c i j) -> p g i c j", g=g, c=C, i=KS, j=KS
                        )
                        for hs in range(g):
                            hl = hg + hs  # chunk-local output row
                            for j in range(KS):
                                src = xt_r[:, hl : hl + KS, j, :]  # (p, i, c)
                                dst = osb_v[:, hs, :, :, j]  # (p, i, c)
                                asm_engines[(hs * KS + j) % len(asm_engines)](
                                    dst, src
                                )
                        st_eng = (
                            nc.scalar if dma_split and (hg // g) % 2 else nc.sync
                        )
                        st_eng.dma_start(
                            out=out_ap[b].rearrange("h w c i j -> w h (c i j)")[
                                :, h0 + hg : h0 + hg + g, :
                            ],
                            in_=osb.rearrange("p (g f) -> p g f", f=C * KS * KS),
                        )

    nc.compile()
    return nc


def _build_kernel_v3(n_b: int = B_LOC, repeat: int = 1, g: int = G, ch: int = CH,
                     xin_bufs: int = 2, ps_bufs: int = 8, osb_bufs: int = 8,
                     load_act: bool = True, asm: str = "vva",
                     store_src: str = "osb", split_store: int = 1):
    """v3: transpose PSUM tiles are copied straight into the interleaved
    output staging buffer (osb), one strided copy per (padded row P ->
    output row h) pair; the xt staging stage is gone.

    Padded row P feeds output rows h in {P-2, P-1, P} (i = h's kernel row
    index = P - h). The copy for (P, h) moves ps (p, j, c) -> osb[h][:,
    c, i, j] as one 3D-AP tensor_copy. A store group of g rows is DMA'd
    once its last contributing row (P = hg_end + 1) has been copied.
    """
    nc = bacc.Bacc("TRN2", target_bir_lowering=False, debug=False)

    x = nc.dram_tensor("x", (n_b, C, H, W), F32, kind="ExternalInput")
    out = nc.dram_tensor("out", (n_b, H, W, C, KS, KS), F32, kind="ExternalOutput")
    x_ap = x.ap()
    out_ap = out.ap()

    with TileContext(nc) as tc:
        with (
            tc.tile_pool(name="const", bufs=1) as const_pool,
            tc.tile_pool(name="xin", bufs=xin_bufs) as xin_pool,
            tc.tile_pool(name="ps", bufs=ps_bufs, space="PSUM") as psum_pool,
            tc.tile_pool(name="osb", bufs=osb_bufs) as out_pool,
        ):
            ident = const_pool.tile([C, C], F32)
            masks.make_identity(nc, ident)
            cst = None
            if store_src == "const":
                cst = const_pool.tile([W, g * C * KS * KS], F32)
                nc.vector.memset(cst, 1.0)

            eng_map = {"v": nc.vector.tensor_copy, "a": nc.scalar.copy}
            asm_engines = [eng_map[c] for c in asm]
            n_asm = 0

            chp = ch + 2
            for _rep in range(repeat):
              for b in range(n_b):
                for h0 in range(0, H, ch):
                    # ---- load chunk: padded rows h0 .. h0+chp-1 ----
                    xin = xin_pool.tile([C, chp * WP], F32)
                    xin_r = xin.rearrange("p (r q) -> p r q", q=WP)
                    nc.vector.memset(xin_r[:, :, 0:1], 0.0)
                    nc.vector.memset(xin_r[:, :, WP - 1 : WP], 0.0)
                    g_lo = h0 - 1
                    lo = 0
                    n_rows = chp
                    if g_lo < 0:
                        nc.vector.memset(xin_r[:, 0:1, :], 0.0)
                        g_lo, lo, n_rows = 0, 1, n_rows - 1
                    if h0 + ch + 1 > H:
                        nc.vector.memset(xin_r[:, chp - 1 : chp, :], 0.0)
                        n_rows -= 1
                    ld_eng = nc.scalar if load_act else nc.sync
                    ld_eng.dma_start(
                        out=xin_r[:, lo : lo + n_rows, 1 : W + 1],
                        in_=x_ap[b, :, g_lo : g_lo + n_rows, :],
                    )

                    # ---- per padded row: transpose, then scatter into osb ----
                    osb_tiles = {}  # chunk-local group idx -> (tile, view)

                    def get_group(gi):
                        if gi not in osb_tiles:
                            t = out_pool.tile([W, g * C * KS * KS], F32)
                            # dims (p, row-in-group, c, i, j)
                            v = t.rearrange(
                                "p (s c i j) -> p s c i j", s=g, c=C, i=KS, j=KS
                            )
                            osb_tiles[gi] = (t, v)
                        return osb_tiles[gi]

                    dst_all = out_ap[b].rearrange("h w c i j -> w h (c i j)")
                    sub = g // split_store

                    for li in range(chp):
                        ps = psum_pool.tile([W, KS * C], F32)
                        for j in range(KS):
                            nc.tensor.transpose(
                                ps[:, j * C : (j + 1) * C],
                                xin_r[:, li, j : j + W],
                                ident,
                            )
                        ps_v = ps.rearrange("p (j c) -> p j c", j=KS)
                        src = ps_v.rearrange("p j c -> p c j")
                        for i in range(KS):
                            hl = li - i  # chunk-local output row
                            if not (0 <= hl < ch):
                                continue
                            gi, s = divmod(hl, g)
                            _, v = get_group(gi)
                            # dst (p, c, j) over osb[h] at fixed i
                            dst = v[:, s, :, i, :]
                            asm_engines[n_asm % len(asm_engines)](dst, src)
                            n_asm += 1
                        # rows [end-sub, end) are complete once li = end+1
                        # has been copied (its i=2 feeds row end-... -> the
                        # last feeder of row r is li = r + 2... wait: row r
                        # gets i=2 from li = r + 2)
                        if li >= sub + 1 and (li - 1) % sub == 0:
                            end = li - 1
                            start = end - sub
                            gi = start // g
                            t, _ = osb_tiles[gi]
                            src_t = cst if cst is not None else t
                            nc.sync.dma_start(
                                out=dst_all[:, h0 + start : h0 + end, :],
                                in_=src_t.rearrange(
                                    "p (s f) -> p s f", f=C * KS * KS
                                )[:, start - gi * g : end - gi * g, :],
                            )
                            if end - gi * g == g:
                                osb_tiles.pop(gi)
                    assert not osb_tiles, osb_tiles

    nc.compile()
    return nc


def _build_kernel_v5(n_b: int = B_LOC, repeat: int = 1, g: int = G, ch: int = CH,
                     xin_bufs: int = 2, ps_bufs: int = 8, osb_bufs: int = 8,
                     load_act: bool = True, asm: str = "vva"):
    """v5: v3's 64-wide transpose/copy pipeline, but the chunk load is a
    single 128-partition (b c) DMA (full 16-engine rate); transposes for
    batch b read lhsT at partition offset b*64.
    """
    assert n_b == 2, n_b
    nc = bacc.Bacc("TRN2", target_bir_lowering=False, debug=False)

    x = nc.dram_tensor("x", (n_b, C, H, W), F32, kind="ExternalInput")
    out = nc.dram_tensor("out", (n_b, H, W, C, KS, KS), F32, kind="ExternalOutput")
    x_bc = x.ap().rearrange("b c h w -> (b c) h w")
    out_ap = out.ap()
    BC = n_b * C

    with TileContext(nc) as tc:
        with (
            tc.tile_pool(name="const", bufs=1) as const_pool,
            tc.tile_pool(name="xin", bufs=xin_bufs) as xin_pool,
            tc.tile_pool(name="ps", bufs=ps_bufs, space="PSUM") as psum_pool,
            tc.tile_pool(name="osb", bufs=osb_bufs) as out_pool,
        ):
            # identity block replicated on both partition halves so the
            # rhs base_partition can match lhsT's (b*C offset)
            ident2 = const_pool.tile([BC, C], F32)
            masks.make_identity(nc, ident2[0:C, :])
            masks.make_identity(nc, ident2[C:BC, :])

            eng_map = {"v": nc.vector.tensor_copy, "a": nc.scalar.copy}
            asm_engines = [eng_map[c] for c in asm]
            n_asm = 0

            chp = ch + 2
            for _rep in range(repeat):
                for h0 in range(0, H, ch):
                    xin = xin_pool.tile([BC, chp * WP], F32)
                    xin_r = xin.rearrange("p (r q) -> p r q", q=WP)
                    nc.vector.memset(xin_r[:, :, 0:1], 0.0)
                    nc.vector.memset(xin_r[:, :, WP - 1 : WP], 0.0)
                    g_lo = h0 - 1
                    lo = 0
                    n_rows = chp
                    if g_lo < 0:
                        nc.vector.memset(xin_r[:, 0:1, :], 0.0)
                        g_lo, lo, n_rows = 0, 1, n_rows - 1
                    if h0 + ch + 1 > H:
                        nc.vector.memset(xin_r[:, chp - 1 : chp, :], 0.0)
                        n_rows -= 1
                    ld_eng = nc.scalar if load_act else nc.sync
                    ld_eng.dma_start(
                        out=xin_r[:, lo : lo + n_rows, 1 : W + 1],
                        in_=x_bc[:, g_lo : g_lo + n_rows, :],
                    )

                    osb_tiles = {}

                    def get_group(b, gi):
                        key = (b, gi)
                        if key not in osb_tiles:
                            t = out_pool.tile([W, g * C * KS * KS], F32)
                            v = t.rearrange(
                                "p (s c i j) -> p s c i j", s=g, c=C, i=KS, j=KS
                            )
                            osb_tiles[key] = (t, v)
                        return osb_tiles[key]

                    def store_group(b, gi):
                        t, _ = osb_tiles.pop((b, gi))
                        nc.sync.dma_start(
                            out=out_ap[b].rearrange("h w c i j -> w h (c i j)")[
                                :, h0 + gi * g : h0 + (gi + 1) * g, :
                            ],
                            in_=t.rearrange("p (s f) -> p s f", f=C * KS * KS),
                        )

                    for li in range(chp):
                        for b in range(n_b):
                            ps = psum_pool.tile([W, KS * C], F32)
                            for j in range(KS):
                                nc.tensor.transpose(
                                    ps[:, j * C : (j + 1) * C],
                                    xin_r[b * C : (b + 1) * C, li, j : j + W],
                                    ident2[b * C : (b + 1) * C, :],
                                )
                            src = ps.rearrange("p (j c) -> p j c", j=KS).rearrange(
                                "p j c -> p c j"
                            )
                            for i in range(KS):
                                hl = li - i
                                if not (0 <= hl < ch):
                                    continue
                                gi, s = divmod(hl, g)
                                _, v = get_group(b, gi)
                                dst = v[:, s, :, i, :]
                                asm_engines[n_asm % len(asm_engines)](dst, src)
                                n_asm += 1
                        if li >= g + 1 and (li - 1) % g == 0:
                            gi = (li - 1) // g - 1
                            for b in range(n_b):
                                store_group(b, gi)
                    assert not osb_tiles, osb_tiles

    nc.compile()
    return nc


def _build_kernel_v4(n_b: int = B_LOC, repeat: int = 1, g: int = G, ch: int = CH,
                     xin_bufs: int = 2, ps_bufs: int = 8, osb_bufs: int = 8,
                     load_act: bool = True, asm: str = "vva"):
    """v4: like v3 (PSUM -> osb direct), but both batches ride one
    128-partition pipeline: the chunk load is a single full-rate DMA with
    partition dim (b c), and each transpose is 128-wide (contraction over
    (b c)), halving TensorE work. Copies then slice the batch halves out
    of PSUM into per-batch osb groups.
    """
    assert n_b == 2, n_b
    nc = bacc.Bacc("TRN2", target_bir_lowering=False, debug=False)

    x = nc.dram_tensor("x", (n_b, C, H, W), F32, kind="ExternalInput")
    out = nc.dram_tensor("out", (n_b, H, W, C, KS, KS), F32, kind="ExternalOutput")
    x_bc = x.ap().rearrange("b c h w -> (b c) h w")
    out_ap = out.ap()
    BC = n_b * C  # 128

    with TileContext(nc) as tc:
        with (
            tc.tile_pool(name="const", bufs=1) as const_pool,
            tc.tile_pool(name="xin", bufs=xin_bufs) as xin_pool,
            tc.tile_pool(name="ps", bufs=ps_bufs, space="PSUM") as psum_pool,
            tc.tile_pool(name="osb", bufs=osb_bufs) as out_pool,
        ):
            ident = const_pool.tile([BC, BC], F32)
            masks.make_identity(nc, ident)

            eng_map = {"v": nc.vector.tensor_copy, "a": nc.scalar.copy}
            asm_engines = [eng_map[c] for c in asm]
            n_asm = 0

            chp = ch + 2
            for _rep in range(repeat):
                for h0 in range(0, H, ch):
                    # ---- one 128-partition load for both batches ----
                    xin = xin_pool.tile([BC, chp * WP], F32)
                    xin_r = xin.rearrange("p (r q) -> p r q", q=WP)
                    nc.vector.memset(xin_r[:, :, 0:1], 0.0)
                    nc.vector.memset(xin_r[:, :, WP - 1 : WP], 0.0)
                    g_lo = h0 - 1
                    lo = 0
                    n_rows = chp
                    if g_lo < 0:
                        nc.vector.memset(xin_r[:, 0:1, :], 0.0)
                        g_lo, lo, n_rows = 0, 1, n_rows - 1
                    if h0 + ch + 1 > H:
                        nc.vector.memset(xin_r[:, chp - 1 : chp, :], 0.0)
                        n_rows -= 1
                    ld_eng = nc.scalar if load_act else nc.sync
                    ld_eng.dma_start(
                        out=xin_r[:, lo : lo + n_rows, 1 : W + 1],
                        in_=x_bc[:, g_lo : g_lo + n_rows, :],
                    )

                    osb_tiles = {}  # (b, chunk-local group idx) -> (tile, view)

                    def get_group(b, gi):
                        key = (b, gi)
                        if key not in osb_tiles:
                            t = out_pool.tile([W, g * C * KS * KS], F32)
                            v = t.rearrange(
                                "p (s c i j) -> p s c i j", s=g, c=C, i=KS, j=KS
                            )
                            osb_tiles[key] = (t, v)
                        return osb_tiles[key]

                    def store_group(b, gi):
                        t, _ = osb_tiles.pop((b, gi))
                        nc.sync.dma_start(
                            out=out_ap[b].rearrange("h w c i j -> w h (c i j)")[
                                :, h0 + gi * g : h0 + (gi + 1) * g, :
                            ],
                            in_=t.rearrange("p (s f) -> p s f", f=C * KS * KS),
                        )

                    for li in range(chp):
                        ps = psum_pool.tile([W, KS * BC], F32)
                        for j in range(KS):
                            nc.tensor.transpose(
                                ps[:, j * BC : (j + 1) * BC],
                                xin_r[:, li, j : j + W],
                                ident,
                            )
                        ps_v = ps.rearrange("p (j bc) -> p j bc", j=KS)
                        for i in range(KS):
                            hl = li - i  # chunk-local output row
                            if not (0 <= hl < ch):
                                continue
                            gi, s = divmod(hl, g)
                            for b in range(n_b):
                                _, v = get_group(b, gi)
                                dst = v[:, s, :, i, :]
                                src = ps_v[:, :, b * C : (b + 1) * C].rearrange(
                                    "p j c -> p c j"
                                )
                                asm_engines[n_asm % len(asm_engines)](dst, src)
                                n_asm += 1
                        if li >= g + 1 and (li - 1) % g == 0:
                            gi = (li - 1) // g - 1
                            for b in range(n_b):
                                store_group(b, gi)
                    assert not osb_tiles, osb_tiles

    nc.compile()
    return nc


FINAL_KW = dict(
    g=4, ch=64, xin_bufs=2, ps_bufs=8, osb_bufs=8, load_act=True, asm="v"
)

_NC_CACHE = {}


def _get_nc(n_b: int):
    if n_b not in _NC_CACHE:
        _NC_CACHE[n_b] = _build_kernel_v3(n_b, **FINAL_KW)
    return _NC_CACHE[n_b]


def run_spmd(x: np.ndarray, **kwargs) -> bass_utils.BassKernelResults:
    """Run the SPMD kernel on 8 cores; returns raw BassKernelResults."""
    x = np.ascontiguousarray(np.asarray(x, dtype=np.float32))
    assert x.shape == (B, C, H, W), x.shape
    nc = _get_nc(B_LOC)
    in_maps = [
        {"x": x[i * B_LOC : (i + 1) * B_LOC]} for i in range(N_CORES)
    ]
    return bass_utils.run_bass_kernel_spmd(
        nc, in_maps, core_ids=list(range(N_CORES)), **kwargs
    )


def kernel(x: np.ndarray) -> np.ndarray:
    res = run_spmd(x)
    return np.concatenate([r["out"] for r in res.results], axis=0)

